# revision 34
# baseline (speedup 1.0000x reference)
"""Trainium2 Bass kernel for nn_BasicResidualBlock (spiking CNN block).

Computation (per reference):
    s1 = IF_scan(x)                 # v += x; s = H(v-1); v *= (1-s)
    y1 = conv3x3(s1, w1) * inv1 + shift1
    s2 = IF_scan(y1)
    out = conv3x3(s2, w2) * inv2 + shift2

Shapes: x [T=8, B=32, C=128, H=32, W=32] fp32.

Strategy:
  - Data-parallel over B across 8 cores (4 images per core).
  - Per (t, b) image: channels C=128 on SBUF partitions, H*W on the free dim.
  - IF neuron state v kept in a zero-padded [128, 34*34] layout so the 3x3
    conv taps can read shifted windows directly (pad border stays exactly 0
    through the IF ops: is_ge(0,1)=0, (0 is_lt 1)*0 = 0).
  - conv3x3 = 9 shifted matmuls accumulating in PSUM; spikes are exact in
    low-precision floats, so products w*s are exact up to weight rounding.
    BN scale is folded into the weights host-side; BN shift is a per-channel
    bias applied on the PSUM->SBUF/v2 path.
  - conv1 weight precision: single fp16 (m10) is NOT enough (spike-threshold
    flips -> rel err 2.96e-2 > 2e-2 gate; bf16 2-split costs 2x PE). Instead:
    fp16 hi split + fp8e4m3 lo split holding the residual scaled by 2^12
    (unscaled residuals ~1e-5 underflow fp8's subnormal floor). The lo split
    runs as fp8 DoubleRow: two taps per instruction (lhsT [C,2,C], rhs
    [C,2,16,32] with the pair dim striding between the two taps' windows),
    9 taps -> 5 paired matmuls per PSUM half at ~1.05x the cost of ONE
    fp16 matmul each. CPU-sim predicts rel err 5.7e-3 (= baseline floor).
  - Scale bookkeeping is free: spikes are stored as {0, 2^-12} (exact in
    fp16 and fp8e5m2) and the fp16 hi weights are pre-scaled by 2^12
    (exact, fits fp16 range), so every product lands at true magnitude in
    the PSUM accumulator. Hi and lo therefore share one PSUM accumulation
    group and v2 += conv1 + shift stays a single DVE op per half.
  - The PE streams fp8 MOVING data ~25% slower than fp16 (273ns vs 220ns
    per 512-row matmul, measured), so spikes live in an fp16 tile for the
    hi/conv2 matmuls with an fp8e5 twin written only for the DoubleRow lo
    pairs (DVE has headroom; PE does not).
  - conv2 feeds the output directly: fp16 weights, single split, {0,1}
    fp16 spikes.
  - Software-pipelined: conv2/output of image i-1 is emitted between conv1
    of image i and i+1 so the PE never waits on the vector-engine IF ops.

Measured: 344.7us HW exec (vs 402.1us bf16-2-split baseline), rel err
5.736e-3. PE busy 330us with <1.2us of gaps — the issue-rate floor for
this tap-decomposed conv (1472 matmuls: fp16 ~215ns, fp8 DoubleRow pair
~244ns). Per-position deltas are flat except a structural +14ns on kx=1
taps (window rows start 2-mod-4 bytes; no row pitch can 4-align all
three kx shifts). Dead ends measured: 1024-wide moving APs (walrus
birverifier rejects >512), DoublePixel perf mode (transparent semantics,
no speedup), fp8/f32r moving data (25%/~2x slower than fp16), output
DMA on the gpsimd queue (NRT_EXEC_UNIT_UNRECOVERABLE crash; the scalar
queue is the safe second DMA queue).
"""

import sys

import numpy as np

try:
    import concourse  # noqa: F401
except ImportError:
    for _p in ("/opt/trn_rl_repo", "/root/.axon_site/_ro/trn_rl_repo"):
        if _p not in sys.path:
            sys.path.insert(0, _p)

import ml_dtypes

EPS = 1e-5
N_CORES = 8
T, B, C, H, W = 8, 32, 128, 32, 32
NB = B // N_CORES          # images per core per timestep
N_IMGS = T * NB            # images per core
HW = H * W                 # 1024
PW = W + 2                 # padded row width 34
PHW = (H + 2) * PW         # 1156
LO_SCALE = 2.0 ** 12       # fp8 lo-split weight scale (spikes carry 2^-12)
# lo-split tap pairing: members share column parity so each pair's windows
# start even-byte-aligned (kx=1 taps read the col-shifted copy1 at kx'=0);
# slots 0-2 pair even-kx taps, slots 3-4 pair the kx=1 taps (+ zero pad)
LO_PAIRS = [(0, 2), (3, 5), (6, 8), (1, 4), (7, 7)]

_program_cache = {}


def build_program(n_imgs=N_IMGS, n_b=NB):
    import bass_rust
    import concourse.mybir as mybir
    from concourse.bacc import Bacc
    from concourse.tile import TileContext

    f32 = mybir.dt.float32
    f16 = mybir.dt.float16
    f8e4 = mybir.dt.float8e4
    f8e5 = mybir.dt.float8e5
    Alu = mybir.AluOpType
    Act = mybir.ActivationFunctionType
    DR = mybir.MatmulPerfMode.DoubleRow
    SPIKE1 = 1.0 / LO_SCALE    # value a live conv1 spike carries

    nc = Bacc()
    x_d = nc.declare_dram_parameter("x", [n_imgs, C, HW], f32, isOutput=False)
    w1h_d = nc.declare_dram_parameter("w1h", [C, 9 * C], f16, isOutput=False)
    w1l_d = nc.declare_dram_parameter("w1l", [C, 10 * C], f8e4, isOutput=False)
    w2_d = nc.declare_dram_parameter("w2", [C, 9 * C], f16, isOutput=False)
    b1_d = nc.declare_dram_parameter("b1", [C, 1], f32, isOutput=False)
    b2_d = nc.declare_dram_parameter("b2", [C, 1], f32, isOutput=False)
    y_d = nc.declare_dram_parameter("y", [n_imgs, C, HW], f32, isOutput=True)

    with TileContext(nc) as tc:
        with (
            tc.tile_pool(name="const", bufs=1) as cp,
            tc.tile_pool(name="state", bufs=1) as vp,
            tc.tile_pool(name="work", bufs=2) as wp,
            tc.tile_pool(name="psum", bufs=4, space="PSUM") as pp,
        ):
            # First input image before the (larger) weight blobs so the
            # startup-critical path (x0 -> IF -> first matmul) isn't queued
            # behind them; memsets go to the otherwise-idle GpSimd engine.
            # x0 arrives in two chunks split at spike row 17 so the first
            # PSUM-half matmuls (needing rows 0..16) start before the full
            # image lands.
            # Each dma_start costs ~650ns of serial issue time on its queue,
            # so the startup-critical transfers are spread over two HWDGE
            # queues: x0 chunks issue on Sync while the w1 blocks issue in
            # parallel on the (otherwise idle until ~28us) Scalar queue.
            X0SPLIT = 17 * W
            x0 = wp.tile([C, HW], f32, tag="xt", bufs=3, name="xt_0")
            w1hs = cp.tile([C, 9 * C], f16, tag="w1hs", name="w1hs")
            w1ls = cp.tile([C, 10 * C], f8e4, tag="w1ls", name="w1ls")
            w2s = cp.tile([C, 9 * C], f16, tag="w2s", name="w2s")
            b1s = cp.tile([C, 1], f32, tag="b1s", name="b1s")
            b2s = cp.tile([C, 1], f32, tag="b2s", name="b2s")
            nc.sync.dma_start(out=x0[:, :X0SPLIT], in_=x_d[0][:, :X0SPLIT])
            nc.scalar.dma_start(out=w1hs, in_=w1h_d[:, :])
            nc.sync.dma_start(out=x0[:, X0SPLIT:], in_=x_d[0][:, X0SPLIT:])
            nc.scalar.dma_start(out=w1ls, in_=w1l_d[:, :])
            nc.scalar.dma_start(out=b1s, in_=b1_d[:, :])
            # w2/b2 are first needed ~25us in (conv2 of image 0); their DMAs
            # are emitted after image 0 so they don't steal HBM bandwidth
            # from the startup-critical x0/w1 transfers.
            def load_conv2_weights():
                nc.scalar.dma_start(out=w2s, in_=w2_d[:, :])
                nc.scalar.dma_start(out=b2s, in_=b2_d[:, :])

            v1 = [vp.tile([C, PHW], f32, tag=f"v1_{b}", name=f"v1_{b}")
                  for b in range(n_b)]
            v2 = [vp.tile([C, PHW], f32, tag=f"v2_{b}", name=f"v2_{b}")
                  for b in range(n_b)]
            # t=0 spike tiles are written interior-only (see below), so zero
            # them fully up front; s1 of image 0 goes first on the GpSimd
            # queue because it gates the very first matmul.
            # fp16 spike tiles hold TWO copies of the padded image: copy0 at
            # [0, PHW) and copy1 (shifted left one column: copy1[r,c] =
            # s[r,c+1]) at [PHW, 2*PHW). kx=1 conv taps read copy1 at kx'=0
            # so their window rows start 4-byte-aligned — misaligned fp16
            # rows cost +14ns/matmul (measured on taps 1/4/7, both halves).
            s1_first = [wp.tile([C, 2 * PHW], f16, tag="s1", bufs=3,
                                name=f"s1_{i}") for i in range(n_b)]
            s8_first = [wp.tile([C, 2 * PHW], f8e5, tag="s8", bufs=3,
                                name=f"s8_{i}") for i in range(n_b)]

            def border_memset(t, base_off=0):
                # Zero only the 1-px pad border (interior is fully written by
                # the t=0 is_ge / reset ops): two strided memsets instead of
                # a full 1156-elem clear — the 16 serial full-tile memsets
                # were the startup critical path (v2acc(0) waited ~25us).
                base = t.rearrange("p (h w) -> p h w", w=PW)[:, 0:2, :]
                p0 = list(base.ap[0])
                rows = base.copy()
                rows.ap = bass_rust.VecI64Pair([p0, [33 * PW, 2], [1, PW]])
                rows.offset = rows.offset + base_off
                cols = base.copy()
                cols.ap = bass_rust.VecI64Pair([p0, [PW, H], [33, 2]])
                cols.offset = cols.offset + PW + base_off
                nc.gpsimd.memset(rows, 0.0)
                nc.gpsimd.memset(cols, 0.0)

            # Image-0-critical tiles first; v2[b] is read-modified by the
            # conv1 accumulate so it needs a full clear.
            for b in range(n_b):
                border_memset(s1_first[b])
                border_memset(s1_first[b], base_off=PHW)
                border_memset(s8_first[b])
                border_memset(s8_first[b], base_off=PHW)
                nc.gpsimd.memset(v2[b], 0.0)
            for b in range(n_b):
                border_memset(v1[b])

            # Warm the PE's HAM clock gate during the startup DMA window with
            # throwaway matmuls on a zeroed tile (cold PE runs at 1.2 GHz for
            # ~3.4us of activity; this burns that ramp on dead time). Scratch
            # PSUM reuses the ps2 slots, which sit idle until ~28us.
            wdum = cp.tile([C, HW // 2], f16, tag="wdum", name="wdum")
            nc.vector.memset(wdum, 0.0)
            for k in range(8):
                psw = pp.tile([C, HW // 2], f32, tag="ps2", bufs=4,
                              name=f"warm_{k}")
                nc.tensor.matmul(out=psw, lhsT=wdum[:, 0:C], rhs=wdum,
                                 start=True, stop=True)

            def if_stage(v, src, s_tile, spike_val, s8_tile=None):
                # v: padded state [C, PHW]; src: [C, HW]; s_tile: [C, 2*PHW]
                vv = v.rearrange("p (h w) -> p h w", w=PW)
                sv = src.rearrange("p (h w) -> p h w", w=W)
                c1v = s_tile[:, PHW:].rearrange("p (h w) -> p h w", w=PW)
                nc.vector.tensor_tensor(
                    out=vv[:, 1:H + 1, 1:W + 1], in0=vv[:, 1:H + 1, 1:W + 1],
                    in1=sv, op=Alu.add)
                nc.vector.tensor_scalar(
                    out=s_tile[:, :PHW], in0=v, scalar1=1.0, scalar2=spike_val,
                    op0=Alu.is_ge, op1=Alu.mult)
                # copy1 = copy0 shifted left one column (v rows 0/33 and col
                # 33 are zero, so borders land zeroed for free)
                nc.vector.tensor_scalar(
                    out=c1v[:, :, 0:PW - 1], in0=vv[:, :, 1:PW],
                    scalar1=1.0, scalar2=spike_val,
                    op0=Alu.is_ge, op1=Alu.mult)
                if s8_tile is not None:
                    c8v = s8_tile[:, PHW:].rearrange("p (h w) -> p h w", w=PW)
                    nc.vector.tensor_scalar(
                        out=s8_tile[:, :PHW], in0=v, scalar1=1.0,
                        scalar2=spike_val, op0=Alu.is_ge, op1=Alu.mult)
                    nc.vector.tensor_scalar(
                        out=c8v[:, :, 0:PW - 1], in0=vv[:, :, 1:PW],
                        scalar1=1.0, scalar2=spike_val,
                        op0=Alu.is_ge, op1=Alu.mult)
                nc.vector.scalar_tensor_tensor(
                    out=v, in0=v, scalar=1.0, in1=v, op0=Alu.is_lt, op1=Alu.mult)

            def lo_rhs(s_tile, h2, tA, tB):
                # Paired moving AP: [C, 2, 16, 32] where dim1 strides between
                # the two taps' shifted windows. kx=1 pairs read copy1 (rows
                # 34+ of the doubled tile) at kx'=0 for even-byte row starts;
                # the pair delta is unaffected (copy offsets cancel).
                kyA, kxA = divmod(tA, 3)
                kyB, kxB = divmod(tB, 3)
                sv = s_tile.rearrange("p (h w) -> p h w", w=PW)
                rbase = 34 if kxA == 1 else 0
                cA = 0 if kxA == 1 else kxA
                r0 = rbase + (H // 2) * h2 + kyA
                base = sv[:, r0:r0 + H // 2, cA:cA + W]
                dlt = ((kyB - kyA) * PW) + (kxB - kxA)
                ap = base.copy()
                p0 = list(ap.ap[0])
                ap.ap = bass_rust.VecI64Pair(
                    [p0, [dlt, 2], [PW, H // 2], [1, W]])
                return ap

            def hi_rhs(s_tile, h2, t):
                # kx=1 taps read copy1 (one-column-left-shifted) at kx'=0 so
                # every window row starts 4-byte-aligned. The [C, 2*PHW]
                # tile viewed as 68 padded rows puts copy1 at row 34.
                sv = s_tile.rearrange("p (h w) -> p h w", w=PW)
                ky, kx = divmod(t, 3)
                r0 = (H // 2) * h2 + ky
                if kx == 1:
                    return sv[:, 34 + r0:34 + r0 + H // 2, 0:W]
                return sv[:, r0:r0 + H // 2, kx:kx + W]

            # Image 0 emits kx=1 taps last so its first matmuls don't wait
            # on the copy1 DVE write (startup-critical path).
            T0_ORDER = [0, 2, 3, 5, 6, 8, 1, 4, 7]

            def conv1(s_tile, s8_tile, psum_tag, halves_inner=True):
                # One PSUM accumulation group per half: 9 fp16 hi matmuls
                # (fp16 spikes) + 5 fp8 DoubleRow lo pairs (fp8e5 twin).
                wv = w1ls.rearrange("p (s two m) -> p s two m", s=5, two=2)
                halves = [pp.tile([C, HW // 2], f32, tag=psum_tag, bufs=4,
                                  name=f"{psum_tag}_{h2}") for h2 in range(2)]
                hi_ts = range(9) if halves_inner else T0_ORDER
                hi = [("h", h2, t) for t in hi_ts for h2 in range(2)]
                lo = [("l", h2, k) for k in range(5) for h2 in range(2)]
                if halves_inner:
                    order = hi + lo
                else:
                    order = ([e for e in hi + lo if e[1] == 0]
                             + [e for e in hi + lo if e[1] == 1])
                for kind, h2, t in order:
                    if kind == "h":
                        nc.tensor.matmul(
                            out=halves[h2],
                            lhsT=w1hs[:, t * C:(t + 1) * C],
                            rhs=hi_rhs(s_tile, h2, t),
                            start=(t == 0), stop=False)
                    else:
                        tA, tB = LO_PAIRS[t]
                        nc.tensor.matmul(
                            out=halves[h2],
                            lhsT=wv[:, t],
                            rhs=lo_rhs(s8_tile, h2, tA, tB),
                            start=False, stop=(t == 4), perf_mode=DR)
                return halves

            def conv2(s_tile, psum_tag="ps2"):
                halves = [pp.tile([C, HW // 2], f32, tag=psum_tag, bufs=4,
                                  name=f"{psum_tag}_{h2}") for h2 in range(2)]
                for t in range(9):
                    for h2 in range(2):
                        nc.tensor.matmul(
                            out=halves[h2],
                            lhsT=w2s[:, t * C:(t + 1) * C],
                            rhs=hi_rhs(s_tile, h2, t),
                            start=(t == 0), stop=(t == 8))
                return halves

            pending = {}
            for i in range(n_imgs + 1):
                if i < n_imgs:
                    b = i % n_b
                    if i == 0:
                        xt = x0
                    else:
                        xt = wp.tile([C, HW], f32, tag="xt", bufs=3,
                                     name=f"xt_{i}")
                        nc.sync.dma_start(out=xt, in_=x_d[i])
                    if i < n_b:
                        # t == 0: v is zero, so spike/reset come straight from
                        # x (skips the accumulate on the startup-critical path;
                        # s border is zeroed by an early gpsimd memset instead
                        # of inherited from the padded v state).
                        s1 = s1_first[i]
                        s8 = s8_first[i]
                        vv = v1[b].rearrange("p (h w) -> p h w", w=PW)
                        xv = xt.rearrange("p (h w) -> p h w", w=W)
                        s1v = s1.rearrange("p (h w) -> p h w", w=PW)
                        s8v = s8.rearrange("p (h w) -> p h w", w=PW)
                        if i == 0:
                            # Split at row 17 to match the x0 DMA chunks:
                            # spikes for PSUM half 0 don't wait on chunk B.
                            # copy1 (rows 34+) is written after copy0/s8 of
                            # each chunk; image 0 orders kx=1 taps last.
                            for r0, r1 in ((0, 17), (17, H)):
                                # order per chunk: copy0 (gates kx!=1 taps),
                                # copy1 (kx=1 taps, ordered 7th+), s8 (lo)
                                for tgt, c0, rb in ((s1v, 1, 1), (s1v, 0, 35),
                                                   (s8v, 1, 1),
                                                   (s8v, 0, 35)):
                                    nc.vector.tensor_scalar(
                                        out=tgt[:, rb + r0:rb + r1,
                                                c0:c0 + W],
                                        in0=xv[:, r0:r1, :],
                                        scalar1=1.0, scalar2=SPIKE1,
                                        op0=Alu.is_ge, op1=Alu.mult)
                        else:
                            for tgt, c0, rb in ((s1v, 1, 1), (s1v, 0, 35),
                                                (s8v, 1, 1), (s8v, 0, 35)):
                                nc.vector.tensor_scalar(
                                    out=tgt[:, rb:rb + H, c0:c0 + W], in0=xv,
                                    scalar1=1.0, scalar2=SPIKE1,
                                    op0=Alu.is_ge, op1=Alu.mult)
                        nc.vector.scalar_tensor_tensor(
                            out=vv[:, 1:H + 1, 1:W + 1], in0=xv, scalar=1.0,
                            in1=xv, op0=Alu.is_lt, op1=Alu.mult)
                    else:
                        s1 = wp.tile([C, 2 * PHW], f16, tag="s1", bufs=3,
                                     name=f"s1_{i}")
                        s8 = wp.tile([C, 2 * PHW], f8e5, tag="s8", bufs=3,
                                     name=f"s8_{i}")
                        if_stage(v1[b], xt, s1, SPIKE1, s8_tile=s8)
                    pending[i] = conv1(s1, s8, "ps1", halves_inner=(i != 0))
                    if i == 0:
                        load_conv2_weights()
                if i >= 1:
                    j = i - 1
                    b = j % n_b
                    ps1 = pending.pop(j)
                    # v2 += conv1_out + shift1, straight from PSUM (one DVE op
                    # per half; no intermediate SBUF copy needed)
                    v2v = v2[b].rearrange("p (h w) -> p h w", w=PW)
                    for h2 in range(2):
                        vint = v2v[:, 1 + (H // 2) * h2:1 + (H // 2) * (h2 + 1),
                                   1:W + 1]
                        nc.vector.scalar_tensor_tensor(
                            out=vint, in0=ps1[h2].rearrange(
                                "p (h w) -> p h w", w=W),
                            scalar=b1s[:, 0:1], in1=vint,
                            op0=Alu.add, op1=Alu.add)
                    s2 = wp.tile([C, 2 * PHW], f16, tag="s2", bufs=3,
                                 name=f"s2_{j}")
                    nc.vector.tensor_scalar(
                        out=s2[:, :PHW], in0=v2[b], scalar1=1.0, scalar2=None,
                        op0=Alu.is_ge)
                    c2v = s2[:, PHW:].rearrange("p (h w) -> p h w", w=PW)
                    nc.vector.tensor_scalar(
                        out=c2v[:, :, 0:PW - 1], in0=v2v[:, :, 1:PW],
                        scalar1=1.0, scalar2=None, op0=Alu.is_ge)
                    nc.vector.scalar_tensor_tensor(
                        out=v2[b], in0=v2[b], scalar=1.0, in1=v2[b],
                        op0=Alu.is_lt, op1=Alu.mult)
                    ps2 = conv2(s2)
                    ot = wp.tile([C, HW], f32, tag="ot", bufs=3, name=f"ot_{j}")
                    for h2 in range(2):
                        sl = slice(h2 * (HW // 2), (h2 + 1) * (HW // 2))
                        nc.scalar.activation(
                            out=ot[:, sl], in_=ps2[h2], func=Act.Identity,
                            bias=b2s[:, 0:1], scale=1.0)
                        # Last image: second half rides the (idle) scalar DMA
                        # queue so the two final transfers overlap across DMA
                        # engines instead of serializing the kernel tail.
                        eng = (nc.scalar if (j == n_imgs - 1 and h2 == 1)
                               else nc.sync)
                        eng.dma_start(out=y_d[j][:, sl], in_=ot[:, sl])

    nc.finalize()
    return nc


def _fold(w, g, b, m, v):
    inv = g.astype(np.float64) / np.sqrt(v.astype(np.float64) + EPS)
    wf = w.astype(np.float64) * inv[:, None, None, None]
    shift = (b.astype(np.float64) - m.astype(np.float64) * inv)
    # [O, I, 3, 3] -> [tap, ci, co]
    lhsT = np.transpose(wf, (2, 3, 1, 0)).reshape(9, C, C)
    return lhsT, shift.astype(np.float32).reshape(C, 1)


def _prep1(w, g, b, m, v):
    lhsT, shift = _fold(w, g, b, m, v)
    hi = lhsT.astype(np.float16)                       # [9, ci, co]
    resid = (lhsT - hi.astype(np.float64)) * LO_SCALE
    lo = resid.astype(ml_dtypes.float8_e4m3)           # [9, ci, co]
    # hi blob pre-scaled by 2^12 (exact in fp16): [ci, tap*C]
    hi_blob = np.ascontiguousarray(
        (hi.astype(np.float64) * LO_SCALE).astype(np.float16)
        .transpose(1, 0, 2).reshape(C, 9 * C))
    # lo blob: [ci, slot, pair, co] -> [ci, 10*C]; slot 4 pair 1 is zero
    lo_sl = np.zeros((5, 2, C, C), ml_dtypes.float8_e4m3)
    for k, (tA, tB) in enumerate(LO_PAIRS):
        lo_sl[k, 0] = lo[tA]
        if tB != tA:
            lo_sl[k, 1] = lo[tB]
    lo_blob = np.ascontiguousarray(
        lo_sl.transpose(2, 0, 1, 3).reshape(C, 10 * C))
    return hi_blob, lo_blob, shift


def _prep2(w, g, b, m, v):
    lhsT, shift = _fold(w, g, b, m, v)
    hi = lhsT.astype(np.float16)
    blob = np.ascontiguousarray(hi.transpose(1, 0, 2).reshape(C, 9 * C))
    return blob, shift


last_results = None  # BassKernelResults of the most recent run (for test.py)

# Note: walrus --enable-ldw-opt=true was tried to elide the redundant weight
# load of each same-lhsT matmul pair; the compiler rejects this kernel's
# Ldweights form ("InstLdweights is not compatible with LDW optimization"),
# so the ~6ns/matmul weight-load issue tax is a hard floor here.


def kernel(x, w1, g1, b1, m1, v1, w2, g2, b2, m2, v2, _trace=False):
    global last_results
    from concourse.bass_utils import run_bass_kernel_spmd

    x = np.asarray(x)
    assert x.shape == (T, B, C, H, W), x.shape

    if "prog" not in _program_cache:
        _program_cache["prog"] = build_program()
    nc = _program_cache["prog"]

    w1h, w1l, sh1 = _prep1(np.asarray(w1), np.asarray(g1), np.asarray(b1),
                           np.asarray(m1), np.asarray(v1))
    w2p, sh2 = _prep2(np.asarray(w2), np.asarray(g2), np.asarray(b2),
                      np.asarray(m2), np.asarray(v2))

    in_maps = []
    for c in range(N_CORES):
        xs = np.ascontiguousarray(
            x[:, c * NB:(c + 1) * NB].reshape(N_IMGS, C, HW))
        in_maps.append({"x": xs, "w1h": w1h, "w1l": w1l, "w2": w2p,
                        "b1": sh1, "b2": sh2})

    last_results = run_bass_kernel_spmd(
        nc, in_maps, list(range(N_CORES)), trace=_trace)
    res = last_results.results
    out = np.empty((T, B, C, H, W), np.float32)
    for c in range(N_CORES):
        out[:, c * NB:(c + 1) * NB] = res[c]["y"].reshape(T, NB, C, H, W)
    return out


# revision 37
# speedup vs baseline: 1.0817x; 1.0817x over previous
"""Trainium2 Bass kernel for nn_BasicResidualBlock (spiking CNN block).

Computation (per reference):
    s1 = IF_scan(x)                 # v += x; s = H(v-1); v *= (1-s)
    y1 = conv3x3(s1, w1) * inv1 + shift1
    s2 = IF_scan(y1)
    out = conv3x3(s2, w2) * inv2 + shift2

Shapes: x [T=8, B=32, C=128, H=32, W=32] fp32.

Strategy:
  - Data-parallel over B across 8 cores (4 images per core).
  - Per (t, b) image: channels C=128 on SBUF partitions, H*W on the free dim.
  - IF neuron state v kept in a zero-padded [128, 34*34] layout so the 3x3
    conv taps can read shifted windows directly (pad border stays exactly 0
    through the IF ops: is_ge(0,1)=0, (0 is_lt 1)*0 = 0).
  - conv3x3 = 9 shifted matmuls accumulating in PSUM; spikes are exact in
    low-precision floats, so products w*s are exact up to weight rounding.
    BN scale is folded into the weights host-side; BN shift is a per-channel
    bias applied on the PSUM->SBUF/v2 path.
  - conv1 weight precision: single fp16 (m10) is NOT enough (spike-threshold
    flips -> rel err 2.96e-2 > 2e-2 gate; bf16 2-split costs 2x PE). Instead:
    fp16 hi split + fp8e4m3 lo split holding the residual scaled by 2^12
    (unscaled residuals ~1e-5 underflow fp8's subnormal floor). The lo split
    runs as fp8 DoubleRow: two taps per instruction (lhsT [C,2,C], rhs
    [C,2,16,32] with the pair dim striding between the two taps' windows),
    9 taps -> 5 paired matmuls per PSUM half at ~1.05x the cost of ONE
    fp16 matmul each. CPU-sim predicts rel err 5.7e-3 (= baseline floor).
  - Scale bookkeeping is free: spikes are stored as {0, 2^-12} (exact in
    fp16 and fp8e5m2) and the fp16 hi weights are pre-scaled by 2^12
    (exact, fits fp16 range), so every product lands at true magnitude in
    the PSUM accumulator. Hi and lo therefore share one PSUM accumulation
    group and v2 += conv1 + shift stays a single DVE op per half.
  - The PE streams fp8 MOVING data ~25% slower than fp16 (273ns vs 220ns
    per 512-row matmul, measured), so spikes live in an fp16 tile for the
    hi/conv2 matmuls with an fp8e5 twin written only for the DoubleRow lo
    pairs (DVE has headroom; PE does not).
  - conv2 feeds the output directly: fp16 weights, single split, {0,1}
    fp16 spikes.
  - Software-pipelined: conv2/output of image i-1 is emitted between conv1
    of image i and i+1 so the PE never waits on the vector-engine IF ops.

Measured: 340.1us HW exec (vs 402.1us bf16-2-split baseline), rel err
5.736e-3, bit-identical to the single-copy variant. PE busy ~325us at
the issue-rate floor for this tap-decomposed conv (fp16 ~215ns/matmul,
fp8 DoubleRow pair ~244ns). The copy1 trick recovered the +14ns/matmul
misalignment tax on fp16 kx=1 taps (window rows at 2-mod-4 bytes); the
same idea applied to the fp8 DR pairs (parity re-pairing, tap deltas
1/32 -> 2/34) measured 27us SLOWER — fp8 pair cost is dominated by the
pair-delta access pattern, not row alignment, so the original pairing
stands. Other dead ends measured: 1024-wide moving APs (walrus rejects
>512), DoublePixel perf mode (no speedup), fp8/f32r moving data
(25%/~2x slower than fp16), output DMA on the gpsimd queue (device
crash; the scalar queue is the safe second DMA queue).
"""

import sys

import numpy as np

try:
    import concourse  # noqa: F401
except ImportError:
    for _p in ("/opt/trn_rl_repo", "/root/.axon_site/_ro/trn_rl_repo"):
        if _p not in sys.path:
            sys.path.insert(0, _p)

import ml_dtypes

EPS = 1e-5
N_CORES = 8
T, B, C, H, W = 8, 32, 128, 32, 32
NB = B // N_CORES          # images per core per timestep
N_IMGS = T * NB            # images per core
HW = H * W                 # 1024
PW = W + 2                 # padded row width 34
PHW = (H + 2) * PW         # 1156
LO_SCALE = 2.0 ** 12       # fp8 lo-split weight scale (spikes carry 2^-12)
# lo-split tap pairing: 9 taps -> 4 pairs + 1 zero-padded pair; column-wise
# pairs give three delta-34 slots (large pair strides measured fastest)
LO_PAIRS = [(0, 3), (1, 4), (2, 5), (6, 7), (8, 8)]

_program_cache = {}


def build_program(n_imgs=N_IMGS, n_b=NB):
    import bass_rust
    import concourse.mybir as mybir
    from concourse.bacc import Bacc
    from concourse.tile import TileContext

    f32 = mybir.dt.float32
    f16 = mybir.dt.float16
    f8e4 = mybir.dt.float8e4
    f8e5 = mybir.dt.float8e5
    Alu = mybir.AluOpType
    Act = mybir.ActivationFunctionType
    DR = mybir.MatmulPerfMode.DoubleRow
    SPIKE1 = 1.0 / LO_SCALE    # value a live conv1 spike carries

    nc = Bacc()
    x_d = nc.declare_dram_parameter("x", [n_imgs, C, HW], f32, isOutput=False)
    w1h_d = nc.declare_dram_parameter("w1h", [C, 9 * C], f16, isOutput=False)
    w1l_d = nc.declare_dram_parameter("w1l", [C, 10 * C], f8e4, isOutput=False)
    w2_d = nc.declare_dram_parameter("w2", [C, 9 * C], f16, isOutput=False)
    b1_d = nc.declare_dram_parameter("b1", [C, 1], f32, isOutput=False)
    b2_d = nc.declare_dram_parameter("b2", [C, 1], f32, isOutput=False)
    y_d = nc.declare_dram_parameter("y", [n_imgs, C, HW], f32, isOutput=True)

    with TileContext(nc) as tc:
        with (
            tc.tile_pool(name="const", bufs=1) as cp,
            tc.tile_pool(name="state", bufs=1) as vp,
            tc.tile_pool(name="work", bufs=2) as wp,
            tc.tile_pool(name="psum", bufs=4, space="PSUM") as pp,
        ):
            # First input image before the (larger) weight blobs so the
            # startup-critical path (x0 -> IF -> first matmul) isn't queued
            # behind them; memsets go to the otherwise-idle GpSimd engine.
            # x0 arrives in two chunks split at spike row 17 so the first
            # PSUM-half matmuls (needing rows 0..16) start before the full
            # image lands.
            # Each dma_start costs ~650ns of serial issue time on its queue,
            # so the startup-critical transfers are spread over two HWDGE
            # queues: x0 chunks issue on Sync while the w1 blocks issue in
            # parallel on the (otherwise idle until ~28us) Scalar queue.
            X0SPLIT = 17 * W
            x0 = wp.tile([C, HW], f32, tag="xt", bufs=3, name="xt_0")
            w1hs = cp.tile([C, 9 * C], f16, tag="w1hs", name="w1hs")
            w1ls = cp.tile([C, 10 * C], f8e4, tag="w1ls", name="w1ls")
            w2s = cp.tile([C, 9 * C], f16, tag="w2s", name="w2s")
            b1s = cp.tile([C, 1], f32, tag="b1s", name="b1s")
            b2s = cp.tile([C, 1], f32, tag="b2s", name="b2s")
            nc.sync.dma_start(out=x0[:, :X0SPLIT], in_=x_d[0][:, :X0SPLIT])
            nc.scalar.dma_start(out=w1hs, in_=w1h_d[:, :])
            nc.sync.dma_start(out=x0[:, X0SPLIT:], in_=x_d[0][:, X0SPLIT:])
            nc.scalar.dma_start(out=w1ls, in_=w1l_d[:, :])
            nc.scalar.dma_start(out=b1s, in_=b1_d[:, :])
            # w2/b2 are first needed ~25us in (conv2 of image 0); their DMAs
            # are emitted after image 0 so they don't steal HBM bandwidth
            # from the startup-critical x0/w1 transfers.
            def load_conv2_weights():
                nc.scalar.dma_start(out=w2s, in_=w2_d[:, :])
                nc.scalar.dma_start(out=b2s, in_=b2_d[:, :])

            v1 = [vp.tile([C, PHW], f32, tag=f"v1_{b}", name=f"v1_{b}")
                  for b in range(n_b)]
            v2 = [vp.tile([C, PHW], f32, tag=f"v2_{b}", name=f"v2_{b}")
                  for b in range(n_b)]
            # t=0 spike tiles are written interior-only (see below), so zero
            # them fully up front; s1 of image 0 goes first on the GpSimd
            # queue because it gates the very first matmul.
            # fp16 spike tiles hold TWO copies of the padded image: copy0 at
            # [0, PHW) and copy1 (shifted left one column: copy1[r,c] =
            # s[r,c+1]) at [PHW, 2*PHW). kx=1 conv taps read copy1 at kx'=0
            # so their window rows start 4-byte-aligned — misaligned fp16
            # rows cost +14ns/matmul (measured on taps 1/4/7, both halves).
            s1_first = [wp.tile([C, 2 * PHW], f16, tag="s1", bufs=3,
                                name=f"s1_{i}") for i in range(n_b)]
            s8_first = [wp.tile([C, PHW], f8e5, tag="s8", bufs=3,
                                name=f"s8_{i}") for i in range(n_b)]

            def border_memset(t, base_off=0):
                # Zero only the 1-px pad border (interior is fully written by
                # the t=0 is_ge / reset ops): two strided memsets instead of
                # a full 1156-elem clear — the 16 serial full-tile memsets
                # were the startup critical path (v2acc(0) waited ~25us).
                base = t.rearrange("p (h w) -> p h w", w=PW)[:, 0:2, :]
                p0 = list(base.ap[0])
                rows = base.copy()
                rows.ap = bass_rust.VecI64Pair([p0, [33 * PW, 2], [1, PW]])
                rows.offset = rows.offset + base_off
                cols = base.copy()
                cols.ap = bass_rust.VecI64Pair([p0, [PW, H], [33, 2]])
                cols.offset = cols.offset + PW + base_off
                nc.gpsimd.memset(rows, 0.0)
                nc.gpsimd.memset(cols, 0.0)

            # Image-0-critical tiles first; v2[b] is read-modified by the
            # conv1 accumulate so it needs a full clear.
            for b in range(n_b):
                border_memset(s1_first[b])
                border_memset(s1_first[b], base_off=PHW)
                border_memset(s8_first[b])
                nc.gpsimd.memset(v2[b], 0.0)
            for b in range(n_b):
                border_memset(v1[b])

            # Warm the PE's HAM clock gate during the startup DMA window with
            # throwaway matmuls on a zeroed tile (cold PE runs at 1.2 GHz for
            # ~3.4us of activity; this burns that ramp on dead time). Scratch
            # PSUM reuses the ps2 slots, which sit idle until ~28us.
            wdum = cp.tile([C, HW // 2], f16, tag="wdum", name="wdum")
            nc.vector.memset(wdum, 0.0)
            for k in range(8):
                psw = pp.tile([C, HW // 2], f32, tag="ps2", bufs=4,
                              name=f"warm_{k}")
                nc.tensor.matmul(out=psw, lhsT=wdum[:, 0:C], rhs=wdum,
                                 start=True, stop=True)

            def if_stage(v, src, s_tile, spike_val, s8_tile=None):
                # v: padded state [C, PHW]; src: [C, HW]; s_tile: [C, 2*PHW]
                vv = v.rearrange("p (h w) -> p h w", w=PW)
                sv = src.rearrange("p (h w) -> p h w", w=W)
                c1v = s_tile[:, PHW:].rearrange("p (h w) -> p h w", w=PW)
                nc.vector.tensor_tensor(
                    out=vv[:, 1:H + 1, 1:W + 1], in0=vv[:, 1:H + 1, 1:W + 1],
                    in1=sv, op=Alu.add)
                nc.vector.tensor_scalar(
                    out=s_tile[:, :PHW], in0=v, scalar1=1.0, scalar2=spike_val,
                    op0=Alu.is_ge, op1=Alu.mult)
                # copy1 = copy0 shifted left one column (v rows 0/33 and col
                # 33 are zero, so borders land zeroed for free)
                nc.vector.tensor_scalar(
                    out=c1v[:, :, 0:PW - 1], in0=vv[:, :, 1:PW],
                    scalar1=1.0, scalar2=spike_val,
                    op0=Alu.is_ge, op1=Alu.mult)
                if s8_tile is not None:
                    nc.vector.tensor_scalar(
                        out=s8_tile, in0=v, scalar1=1.0, scalar2=spike_val,
                        op0=Alu.is_ge, op1=Alu.mult)
                nc.vector.scalar_tensor_tensor(
                    out=v, in0=v, scalar=1.0, in1=v, op0=Alu.is_lt, op1=Alu.mult)

            def lo_rhs(s_tile, h2, tA, tB):
                # Paired moving AP: [C, 2, 16, 32] where dim1 strides between
                # the two taps' shifted windows within the padded spike tile.
                kyA, kxA = divmod(tA, 3)
                kyB, kxB = divmod(tB, 3)
                sv = s_tile.rearrange("p (h w) -> p h w", w=PW)
                r0 = (H // 2) * h2 + kyA
                base = sv[:, r0:r0 + H // 2, kxA:kxA + W]
                dlt = ((kyB - kyA) * PW) + (kxB - kxA)
                ap = base.copy()
                p0 = list(ap.ap[0])
                ap.ap = bass_rust.VecI64Pair(
                    [p0, [dlt, 2], [PW, H // 2], [1, W]])
                return ap

            def hi_rhs(s_tile, h2, t):
                # kx=1 taps read copy1 (one-column-left-shifted) at kx'=0 so
                # every window row starts 4-byte-aligned. The [C, 2*PHW]
                # tile viewed as 68 padded rows puts copy1 at row 34.
                sv = s_tile.rearrange("p (h w) -> p h w", w=PW)
                ky, kx = divmod(t, 3)
                r0 = (H // 2) * h2 + ky
                if kx == 1:
                    return sv[:, 34 + r0:34 + r0 + H // 2, 0:W]
                return sv[:, r0:r0 + H // 2, kx:kx + W]

            # Image 0 emits kx=1 taps last so its first matmuls don't wait
            # on the copy1 DVE write (startup-critical path).
            T0_ORDER = [0, 2, 3, 5, 6, 8, 1, 4, 7]

            def conv1(s_tile, s8_tile, psum_tag, halves_inner=True):
                # One PSUM accumulation group per half: 9 fp16 hi matmuls
                # (fp16 spikes) + 5 fp8 DoubleRow lo pairs (fp8e5 twin).
                wv = w1ls.rearrange("p (s two m) -> p s two m", s=5, two=2)
                halves = [pp.tile([C, HW // 2], f32, tag=psum_tag, bufs=4,
                                  name=f"{psum_tag}_{h2}") for h2 in range(2)]
                hi_ts = range(9) if halves_inner else T0_ORDER
                hi = [("h", h2, t) for t in hi_ts for h2 in range(2)]
                lo = [("l", h2, k) for k in range(5) for h2 in range(2)]
                if halves_inner:
                    order = hi + lo
                else:
                    order = ([e for e in hi + lo if e[1] == 0]
                             + [e for e in hi + lo if e[1] == 1])
                for kind, h2, t in order:
                    if kind == "h":
                        nc.tensor.matmul(
                            out=halves[h2],
                            lhsT=w1hs[:, t * C:(t + 1) * C],
                            rhs=hi_rhs(s_tile, h2, t),
                            start=(t == 0), stop=False)
                    else:
                        tA, tB = LO_PAIRS[t]
                        nc.tensor.matmul(
                            out=halves[h2],
                            lhsT=wv[:, t],
                            rhs=lo_rhs(s8_tile, h2, tA, tB),
                            start=False, stop=(t == 4), perf_mode=DR)
                return halves

            def conv2(s_tile, psum_tag="ps2"):
                halves = [pp.tile([C, HW // 2], f32, tag=psum_tag, bufs=4,
                                  name=f"{psum_tag}_{h2}") for h2 in range(2)]
                for t in range(9):
                    for h2 in range(2):
                        nc.tensor.matmul(
                            out=halves[h2],
                            lhsT=w2s[:, t * C:(t + 1) * C],
                            rhs=hi_rhs(s_tile, h2, t),
                            start=(t == 0), stop=(t == 8))
                return halves

            pending = {}
            for i in range(n_imgs + 1):
                if i < n_imgs:
                    b = i % n_b
                    if i == 0:
                        xt = x0
                    else:
                        xt = wp.tile([C, HW], f32, tag="xt", bufs=3,
                                     name=f"xt_{i}")
                        nc.sync.dma_start(out=xt, in_=x_d[i])
                    if i < n_b:
                        # t == 0: v is zero, so spike/reset come straight from
                        # x (skips the accumulate on the startup-critical path;
                        # s border is zeroed by an early gpsimd memset instead
                        # of inherited from the padded v state).
                        s1 = s1_first[i]
                        s8 = s8_first[i]
                        vv = v1[b].rearrange("p (h w) -> p h w", w=PW)
                        xv = xt.rearrange("p (h w) -> p h w", w=W)
                        s1v = s1.rearrange("p (h w) -> p h w", w=PW)
                        s8v = s8.rearrange("p (h w) -> p h w", w=PW)
                        if i == 0:
                            # Split at row 17 to match the x0 DMA chunks:
                            # spikes for PSUM half 0 don't wait on chunk B.
                            # copy1 (rows 34+) is written after copy0/s8 of
                            # each chunk; image 0 orders kx=1 taps last.
                            for r0, r1 in ((0, 17), (17, H)):
                                # order per chunk: copy0 (gates kx!=1 taps),
                                # copy1 (kx=1 taps, ordered 7th+), s8 (lo)
                                for tgt, c0, rb in ((s1v, 1, 1), (s1v, 0, 35),
                                                   (s8v, 1, 1)):
                                    nc.vector.tensor_scalar(
                                        out=tgt[:, rb + r0:rb + r1,
                                                c0:c0 + W],
                                        in0=xv[:, r0:r1, :],
                                        scalar1=1.0, scalar2=SPIKE1,
                                        op0=Alu.is_ge, op1=Alu.mult)
                        else:
                            for tgt, c0 in ((s1v, 1), (s8v, 1)):
                                nc.vector.tensor_scalar(
                                    out=tgt[:, 1:H + 1, c0:c0 + W], in0=xv,
                                    scalar1=1.0, scalar2=SPIKE1,
                                    op0=Alu.is_ge, op1=Alu.mult)
                            nc.vector.tensor_scalar(
                                out=s1v[:, 35:35 + H, 0:W], in0=xv,
                                scalar1=1.0, scalar2=SPIKE1,
                                op0=Alu.is_ge, op1=Alu.mult)
                        nc.vector.scalar_tensor_tensor(
                            out=vv[:, 1:H + 1, 1:W + 1], in0=xv, scalar=1.0,
                            in1=xv, op0=Alu.is_lt, op1=Alu.mult)
                    else:
                        s1 = wp.tile([C, 2 * PHW], f16, tag="s1", bufs=3,
                                     name=f"s1_{i}")
                        s8 = wp.tile([C, PHW], f8e5, tag="s8", bufs=3,
                                     name=f"s8_{i}")
                        if_stage(v1[b], xt, s1, SPIKE1, s8_tile=s8)
                    pending[i] = conv1(s1, s8, "ps1", halves_inner=(i != 0))
                    if i == 0:
                        load_conv2_weights()
                if i >= 1:
                    j = i - 1
                    b = j % n_b
                    ps1 = pending.pop(j)
                    # v2 += conv1_out + shift1, straight from PSUM (one DVE op
                    # per half; no intermediate SBUF copy needed)
                    v2v = v2[b].rearrange("p (h w) -> p h w", w=PW)
                    for h2 in range(2):
                        vint = v2v[:, 1 + (H // 2) * h2:1 + (H // 2) * (h2 + 1),
                                   1:W + 1]
                        nc.vector.scalar_tensor_tensor(
                            out=vint, in0=ps1[h2].rearrange(
                                "p (h w) -> p h w", w=W),
                            scalar=b1s[:, 0:1], in1=vint,
                            op0=Alu.add, op1=Alu.add)
                    s2 = wp.tile([C, 2 * PHW], f16, tag="s2", bufs=3,
                                 name=f"s2_{j}")
                    nc.vector.tensor_scalar(
                        out=s2[:, :PHW], in0=v2[b], scalar1=1.0, scalar2=None,
                        op0=Alu.is_ge)
                    c2v = s2[:, PHW:].rearrange("p (h w) -> p h w", w=PW)
                    nc.vector.tensor_scalar(
                        out=c2v[:, :, 0:PW - 1], in0=v2v[:, :, 1:PW],
                        scalar1=1.0, scalar2=None, op0=Alu.is_ge)
                    nc.vector.scalar_tensor_tensor(
                        out=v2[b], in0=v2[b], scalar=1.0, in1=v2[b],
                        op0=Alu.is_lt, op1=Alu.mult)
                    ps2 = conv2(s2)
                    ot = wp.tile([C, HW], f32, tag="ot", bufs=3, name=f"ot_{j}")
                    for h2 in range(2):
                        sl = slice(h2 * (HW // 2), (h2 + 1) * (HW // 2))
                        nc.scalar.activation(
                            out=ot[:, sl], in_=ps2[h2], func=Act.Identity,
                            bias=b2s[:, 0:1], scale=1.0)
                        # Last image: second half rides the (idle) scalar DMA
                        # queue so the two final transfers overlap across DMA
                        # engines instead of serializing the kernel tail.
                        eng = (nc.scalar if (j == n_imgs - 1 and h2 == 1)
                               else nc.sync)
                        eng.dma_start(out=y_d[j][:, sl], in_=ot[:, sl])

    nc.finalize()
    return nc


def _fold(w, g, b, m, v):
    inv = g.astype(np.float64) / np.sqrt(v.astype(np.float64) + EPS)
    wf = w.astype(np.float64) * inv[:, None, None, None]
    shift = (b.astype(np.float64) - m.astype(np.float64) * inv)
    # [O, I, 3, 3] -> [tap, ci, co]
    lhsT = np.transpose(wf, (2, 3, 1, 0)).reshape(9, C, C)
    return lhsT, shift.astype(np.float32).reshape(C, 1)


def _prep1(w, g, b, m, v):
    lhsT, shift = _fold(w, g, b, m, v)
    hi = lhsT.astype(np.float16)                       # [9, ci, co]
    resid = (lhsT - hi.astype(np.float64)) * LO_SCALE
    lo = resid.astype(ml_dtypes.float8_e4m3)           # [9, ci, co]
    # hi blob pre-scaled by 2^12 (exact in fp16): [ci, tap*C]
    hi_blob = np.ascontiguousarray(
        (hi.astype(np.float64) * LO_SCALE).astype(np.float16)
        .transpose(1, 0, 2).reshape(C, 9 * C))
    # lo blob: [ci, slot, pair, co] -> [ci, 10*C]; slot 4 pair 1 is zero
    lo_sl = np.zeros((5, 2, C, C), ml_dtypes.float8_e4m3)
    for k, (tA, tB) in enumerate(LO_PAIRS):
        lo_sl[k, 0] = lo[tA]
        if tB != tA:
            lo_sl[k, 1] = lo[tB]
    lo_blob = np.ascontiguousarray(
        lo_sl.transpose(2, 0, 1, 3).reshape(C, 10 * C))
    return hi_blob, lo_blob, shift


def _prep2(w, g, b, m, v):
    lhsT, shift = _fold(w, g, b, m, v)
    hi = lhsT.astype(np.float16)
    blob = np.ascontiguousarray(hi.transpose(1, 0, 2).reshape(C, 9 * C))
    return blob, shift


last_results = None  # BassKernelResults of the most recent run (for test.py)

# Note: walrus --enable-ldw-opt=true was tried to elide the redundant weight
# load of each same-lhsT matmul pair; the compiler rejects this kernel's
# Ldweights form ("InstLdweights is not compatible with LDW optimization"),
# so the ~6ns/matmul weight-load issue tax is a hard floor here.


def kernel(x, w1, g1, b1, m1, v1, w2, g2, b2, m2, v2, _trace=False):
    global last_results
    from concourse.bass_utils import run_bass_kernel_spmd

    x = np.asarray(x)
    assert x.shape == (T, B, C, H, W), x.shape

    if "prog" not in _program_cache:
        _program_cache["prog"] = build_program()
    nc = _program_cache["prog"]

    w1h, w1l, sh1 = _prep1(np.asarray(w1), np.asarray(g1), np.asarray(b1),
                           np.asarray(m1), np.asarray(v1))
    w2p, sh2 = _prep2(np.asarray(w2), np.asarray(g2), np.asarray(b2),
                      np.asarray(m2), np.asarray(v2))

    in_maps = []
    for c in range(N_CORES):
        xs = np.ascontiguousarray(
            x[:, c * NB:(c + 1) * NB].reshape(N_IMGS, C, HW))
        in_maps.append({"x": xs, "w1h": w1h, "w1l": w1l, "w2": w2p,
                        "b1": sh1, "b2": sh2})

    last_results = run_bass_kernel_spmd(
        nc, in_maps, list(range(N_CORES)), trace=_trace)
    res = last_results.results
    out = np.empty((T, B, C, H, W), np.float32)
    for c in range(N_CORES):
        out[:, c * NB:(c + 1) * NB] = res[c]["y"].reshape(T, NB, C, H, W)
    return out
